# revision 2
# baseline (speedup 1.0000x reference)
"""BoundaryDoULoss Trainium2 kernel, v2.

Data-parallel over batch: 16 images sharded 2-per-core across 8 NeuronCores;
each core computes per-class partial sums (S region count, N interior count,
I = sum(p*onehot), Z = sum(p^2)) over its shard; the host reduces partials,
forms C = S - N, alpha, and the scalar loss.

Layout per image: [512, 512] -> [128 partitions, 4 rows x 512 cols free].
The host pre-packs inputs per-partition-contiguous; logits travel as
fp8-e4m3 (the softmax bit-hack absorbs the quantization noise) and are
loaded in four 512-column quarters per image so DMA, ACT, DVE and PE
pipeline at quarter granularity.

Softmax runs in bf16 bit-space:
  i_e = int16(x * 128*log2(e) + 16252)      # Schraudolph exp (ACT affine)
  se  = (e0+e2)+(e1+e3)                     # bf16 adds (DVE 2x)
  i_w = i_e + (16248.75 - bitcast16(se))    # log-domain divide (DVE)
The exp/divide sawtooth biases cancel to ~2e-4 in the final loss (validated
against the fp64 reference; tolerance is 2e-2).

Boundary detection uses a host-side encoding q = 4^t in {1,4,16,64}: sums of
four such values are unique per multiset and integer-exact in bf16, so a
pixel is interior iff (up+down+left+right) == 4q. A 448 sentinel row feeds
the image-top/bottom halos, and border columns are forced non-interior with
memsets. The halo row sums run on Pool; the combine + compare runs on DVE.

Per-class reductions (I, N, Z) run on the otherwise-idle PE as chunked
diagonal accumulations in PSUM (one lazy-zero accumulation group per bank);
ACT bounces the banks to SBUF as bf16 and they are DMA'd out raw - the host
sums the 128x128 block diagonals.
"""

import numpy as np
import ml_dtypes
import concourse.tile as tile
import concourse.mybir as mybir
from concourse import bacc
from concourse.bass_utils import run_bass_kernel_spmd

N_CORES = 8
B, NCLS, H, W = 16, 4, 512, 512
BL = B // N_CORES  # images per core
R = 4  # rows per partition
P = 128
FW = R * W  # free size of one image tile
NQ = 4  # column quarters per image
QF = FW // NQ  # quarter free size (512)
SMOOTH = 1e-5

A_EXP = float(np.float32(128.0 / np.log(2.0)))  # 184.66162
B_EXP = 16256.0 - 4.0
K_DIV = 16256.0 - 7.25
SENTINEL = 448.0
QVALS = (1.0, 4.0, 16.0, 64.0)
QW = FW + 2 * W  # q-pack free size: q rows + hup + hdn

f32 = mybir.dt.float32
bf16 = mybir.dt.bfloat16
i16 = mybir.dt.int16
fp8 = mybir.dt.float8e4
Alu = mybir.AluOpType
AF = mybir.ActivationFunctionType

_cache = {}


def _kernel_body(nc, tc, x_ap, q_ap, out_ap, dump_ap):
    with (
        tc.tile_pool(name="io", bufs=2) as io_pool,
        tc.tile_pool(name="work", bufs=2) as work_pool,
        tc.tile_pool(name="acc", bufs=1) as acc_pool,
        tc.tile_pool(name="ps", bufs=2, space="PSUM") as psum_pool,
    ):
        st_s = acc_pool.tile([P, 2 * NCLS], f32)  # DVE-accumulated S counts

        for b in range(BL):
            # ---- packed DMAs on one FIFO: q+halos, then 4 logit quarters
            qt = io_pool.tile([P, QW], bf16, tag="qt")
            nc.sync.dma_start(qt[:], q_ap[b])
            xq = []
            for qq in range(NQ):
                xt = io_pool.tile([P, NCLS, QF], fp8, tag=f"xt{qq}")
                nc.sync.dma_start(xt[:].rearrange("p c n -> p (c n)"), x_ap[b, qq])
                xq.append(xt)

            qf = qt[:, 0:FW]
            q3 = qf.rearrange("p (r w) -> p r w", r=R)
            hup = qt[:, FW : FW + W]
            hdn = qt[:, FW + W : FW + 2 * W]

            # ---- one-hots + S (DVE 4x) - ready as soon as q lands
            o = NCLS * b
            oh = work_pool.tile([P, NCLS, FW], bf16, tag="oh")
            for c in range(NCLS):
                nc.vector.tensor_scalar(
                    oh[:, c], qf, QVALS[c], None, op0=Alu.is_equal, op1=Alu.add,
                    accum_out=st_s[:, o + c : o + c + 1],
                )
            q4 = work_pool.tile([P, FW], bf16, tag="q4")
            nc.vector.tensor_scalar(q4[:], qf, 4.0, None, op0=Alu.mult)

            # ---- halo row sums: vertical on DVE (fast, keeps the combine
            # ready early), horizontal on Pool
            v = work_pool.tile([P, R, W], bf16, tag="v")
            nc.vector.tensor_tensor(
                v[:, 1:3, :], q3[:, 0:2, :], q3[:, 2:4, :], op=Alu.add
            )
            nc.vector.tensor_tensor(v[:, 0, :], hup, q3[:, 1, :], op=Alu.add)
            nc.vector.tensor_tensor(v[:, 3, :], q3[:, 2, :], hdn, op=Alu.add)
            h = work_pool.tile([P, FW], bf16, tag="h")
            nc.vector.memset(h[:, 0:1], SENTINEL)
            nc.vector.memset(h[:, FW - 1 : FW], SENTINEL)
            nc.gpsimd.tensor_tensor(
                h[:, 1 : FW - 1], qf[:, 0 : FW - 2], qf[:, 2:FW], op=Alu.add
            )

            ie = work_pool.tile([P, NCLS, FW], i16, tag="ie")
            s02 = work_pool.tile([P, 2, FW], bf16, tag="s02")
            se = work_pool.tile([P, FW], bf16, tag="se")
            nse = work_pool.tile([P, FW], i16, tag="nse")
            e = ie[:].bitcast(bf16)
            w = ie[:].bitcast(bf16)
            int_m = work_pool.tile([P, FW], bf16, tag="int_m")
            psum_iz = psum_pool.tile([P, 2, NCLS, 128], f32, tag="piz")
            psum_i = psum_iz[:, 0]
            psum_z = psum_iz[:, 1]
            psum_n = psum_pool.tile([P, NCLS, 128], f32, tag="pn")
            cp = work_pool.tile([P, 3, NCLS * 128], bf16, tag="cp")

            def softmax_quarter(qq):
                qa = slice(qq * QF, (qq + 1) * QF)
                nc.scalar.activation(
                    ie[:, :, qa], xq[qq][:], AF.Copy, bias=B_EXP, scale=A_EXP
                )
                nc.vector.tensor_tensor(
                    s02[:, :, qa], e[:, 0:2, qa], e[:, 2:4, qa], op=Alu.add
                )
                nc.vector.tensor_tensor(
                    se[:, qa], s02[:, 0, qa], s02[:, 1, qa], op=Alu.add
                )
                nc.vector.tensor_scalar(
                    nse[:, qa], se[:, qa].bitcast(i16), -1.0, K_DIV,
                    op0=Alu.mult, op1=Alu.add,
                )
                nse_b = nse[:, qa].unsqueeze(1).broadcast_to((P, NCLS, QF))
                nc.vector.tensor_tensor(ie[:, :, qa], ie[:, :, qa], nse_b, op=Alu.add)

            def iz_quarter(qq):
                # one lazy-zero accumulation group per PSUM bank: only the
                # first matmul starts it, only the very last stops it
                first = qq == 0
                last = qq == NQ - 1
                for c in range(NCLS):
                    for ch in range(QF // 128):
                        sl = slice(qq * QF + ch * 128, qq * QF + (ch + 1) * 128)
                        nc.tensor.matmul(
                            psum_i[:, c], w[:, c, sl], oh[:, c, sl],
                            start=(first and c == 0 and ch == 0),
                            stop=(last and c == NCLS - 1 and ch == QF // 128 - 1),
                        )
                    for ch in range(QF // 128):
                        sl = slice(qq * QF + ch * 128, qq * QF + (ch + 1) * 128)
                        nc.tensor.matmul(
                            psum_z[:, c], w[:, c, sl], w[:, c, sl],
                            start=(first and c == 0 and ch == 0),
                            stop=(last and c == NCLS - 1 and ch == QF // 128 - 1),
                        )

            for qq in range(NQ):
                softmax_quarter(qq)
                iz_quarter(qq)
                if qq == 1:
                    # interior mask combine on Pool, compare on DVE
                    nc.gpsimd.tensor_tensor(
                        h[:], h[:], v[:].rearrange("p r w -> p (r w)"), op=Alu.add
                    )
                    nc.vector.tensor_tensor(
                        int_m[:], h[:], q4[:], op=Alu.is_equal
                    )
                    i3 = int_m[:].rearrange("p (r w) -> p r w", r=R)
                    nc.vector.memset(i3[:, :, 0:1], 0.0)
                    nc.vector.memset(i3[:, :, W - 1 : W], 0.0)
                if qq == 1:
                    # N-chains as soon as the interior mask is ready
                    for c in range(NCLS):
                        for ch in range(FW // 128):
                            sl = slice(ch * 128, (ch + 1) * 128)
                            nc.tensor.matmul(
                                psum_n[:, c], oh[:, c, sl], int_m[:, sl],
                                start=(c == 0 and ch == 0),
                                stop=(c == NCLS - 1 and ch == FW // 128 - 1),
                            )
                    nc.scalar.activation(
                        cp[:, 2], psum_n[:].rearrange("p c n -> p (c n)"), AF.Copy
                    )
                    nc.sync.dma_start(
                        dump_ap[b, :, 2 * NCLS * 128 :], cp[:, 2]
                    )

            # bounce I/Z PSUM to SBUF as bf16 in one op and dump; the host
            # sums the block diagonals (dump layout: [I, N, Z] -> I at 0,
            # Z written right after I here, N separate)
            izv = cp[:, 0:2].rearrange("p f n -> p (f n)")
            nc.scalar.activation(
                izv, psum_iz[:].rearrange("p f c n -> p (f c n)"), AF.Copy
            )
            nc.sync.dma_start(dump_ap[b, :, 0 : 2 * NCLS * 128], izv)

        nc.sync.dma_start(out_ap[:], st_s[:])


def _build():
    if "nc" in _cache:
        return _cache["nc"]
    nc = bacc.Bacc("TRN2", target_bir_lowering=False, debug=False, num_devices=N_CORES)
    x_ap = nc.dram_tensor("x", [BL, NQ, P, NCLS * QF], fp8, kind="ExternalInput").ap()
    q_ap = nc.dram_tensor("q", [BL, P, QW], bf16, kind="ExternalInput").ap()
    out_ap = nc.dram_tensor("stats", [P, 2 * NCLS], f32, kind="ExternalOutput").ap()
    dump_ap = nc.dram_tensor(
        "dumps", [BL, P, 3 * NCLS * 128], bf16, kind="ExternalOutput"
    ).ap()
    with tile.TileContext(nc) as tc:
        _kernel_body(nc, tc, x_ap, q_ap, out_ap, dump_ap)
    nc.compile()
    _cache["nc"] = nc
    return nc


def _host_inputs(inputs, target):
    """Pack per-core inputs into per-partition-contiguous layouts."""
    nb = inputs.shape[0]
    x8 = inputs.astype(ml_dtypes.float8_e4m3)
    # [b, c, (p qq), w] -> [b, qq, p, c, w]   (quarter qq = row 4p+qq)
    xp = np.ascontiguousarray(
        x8.reshape(nb, NCLS, P, NQ, W).transpose(0, 3, 2, 1, 4).reshape(
            nb, NQ, P, NCLS * QF
        )
    )
    q = np.power(4.0, target.astype(np.float32)).astype(ml_dtypes.bfloat16)
    qp = np.empty((nb, P, QW), dtype=ml_dtypes.bfloat16)
    qp[:, :, 0:FW] = q.reshape(nb, P, FW)
    qp[:, 1:, FW : FW + W] = q.reshape(nb, H, W)[:, R - 1 : H - 1 : R]  # hup
    qp[:, 0, FW : FW + W] = SENTINEL
    qp[:, : P - 1, FW + W :] = q.reshape(nb, H, W)[:, R : H : R]  # hdn
    qp[:, P - 1, FW + W :] = SENTINEL
    return xp, qp


_DIAG = np.arange(P)


def _diag_sums(dump):
    """dump: [P, 3*NCLS*128] -> [3, NCLS] diagonal sums."""
    d = dump.reshape(P, 3, NCLS, 128)
    izn = d[_DIAG, :, :, _DIAG].sum(axis=0)  # rows [I, Z, N]
    return izn[[0, 2, 1]]  # -> [I, N, Z]


def _finish(S, I, N, Z):
    C = S - N
    alpha = 1.0 - (C + SMOOTH) / (S + SMOOTH)
    alpha = np.minimum(2.0 * alpha - 1.0, 0.8)
    loss_c = (Z + S - 2.0 * I + SMOOTH) / (Z + S - (1.0 + alpha) * I + SMOOTH)
    return np.float32(loss_c.mean())


def kernel(inputs: np.ndarray, target: np.ndarray) -> np.ndarray:
    nc = _build()
    xp, qp = _host_inputs(inputs, target)
    in_maps = [
        {"x": xp[c * BL : (c + 1) * BL], "q": qp[c * BL : (c + 1) * BL]}
        for c in range(N_CORES)
    ]
    for attempt in range(3):
        res = run_bass_kernel_spmd(nc, in_maps, list(range(N_CORES)))
        S = np.zeros(NCLS)
        INZ = np.zeros((3, NCLS))
        ok = True
        for c in range(N_CORES):
            st = res.results[c]["stats"].astype(np.float64)
            dumps = res.results[c]["dumps"].astype(np.float64)
            ok &= bool(np.isfinite(st).all() and np.isfinite(dumps).all())
            S += st.sum(axis=0).reshape(BL, NCLS).sum(axis=0)
            for b in range(BL):
                INZ += _diag_sums(dumps[b])
        # S counts must equal the pixel total; retry on transient faults
        if ok and abs(S.sum() - B * H * W) < 0.5:
            break
    I, N, Z = INZ
    return _finish(S, I, N, Z)


# revision 4
# speedup vs baseline: 1.0057x; 1.0057x over previous
"""BoundaryDoULoss Trainium2 kernel, v2.

Data-parallel over batch: 16 images sharded 2-per-core across 8 NeuronCores;
each core computes per-class partial sums (S region count, N interior count,
I = sum(p*onehot), Z = sum(p^2)) over its shard; the host reduces partials,
forms C = S - N, alpha, and the scalar loss.

Layout per image: [512, 512] -> [128 partitions, 4 rows x 512 cols free].
The host pre-packs inputs per-partition-contiguous; logits travel as
fp8-e4m3 (the softmax bit-hack absorbs the quantization noise) and are
loaded in four 512-column quarters per image so DMA, ACT, DVE and PE
pipeline at quarter granularity.

Softmax runs in bf16 bit-space:
  i_e = int16(x * 128*log2(e) + 16252)      # Schraudolph exp (ACT affine)
  se  = (e0+e2)+(e1+e3)                     # bf16 adds (DVE 2x)
  i_w = i_e + (16248.75 - bitcast16(se))    # log-domain divide (DVE)
The exp/divide sawtooth biases cancel to ~2e-4 in the final loss (validated
against the fp64 reference; tolerance is 2e-2).

Boundary detection uses a host-side encoding q = 4^t in {1,4,16,64}: sums of
four such values are unique per multiset and integer-exact in bf16, so a
pixel is interior iff (up+down+left+right) == 4q. A 448 sentinel row feeds
the image-top/bottom halos, and border columns are forced non-interior with
memsets. The halo row sums run on Pool; the combine + compare runs on DVE.

Per-class reductions (I, N, Z) run on the otherwise-idle PE as chunked
diagonal accumulations in PSUM (one lazy-zero accumulation group per bank);
ACT bounces the banks to SBUF as bf16 and they are DMA'd out raw - the host
sums the 128x128 block diagonals.
"""

import numpy as np
import ml_dtypes
import concourse.tile as tile
import concourse.mybir as mybir
from concourse import bacc
from concourse.bass_utils import run_bass_kernel_spmd

N_CORES = 8
B, NCLS, H, W = 16, 4, 512, 512
BL = B // N_CORES  # images per core
R = 4  # rows per partition
P = 128
FW = R * W  # free size of one image tile
NQ = 4  # column quarters per image
QF = FW // NQ  # quarter free size (512)
SMOOTH = 1e-5

A_EXP = float(np.float32(128.0 / np.log(2.0)))  # 184.66162
B_EXP = 16256.0 - 4.0
K_DIV = 16256.0 - 7.25
SENTINEL = 448.0
QVALS = (1.0, 4.0, 16.0, 64.0)
QW = FW + 2 * W  # q-pack free size: q rows + hup + hdn

f32 = mybir.dt.float32
bf16 = mybir.dt.bfloat16
i16 = mybir.dt.int16
fp8 = mybir.dt.float8e4
Alu = mybir.AluOpType
AF = mybir.ActivationFunctionType

_cache = {}


def _kernel_body(nc, tc, x_ap, q_ap, out_ap, dump_ap):
    with (
        tc.tile_pool(name="io", bufs=2) as io_pool,
        tc.tile_pool(name="work", bufs=2) as work_pool,
        tc.tile_pool(name="acc", bufs=1) as acc_pool,
        tc.tile_pool(name="ps", bufs=2, space="PSUM") as psum_pool,
    ):
        st_s = acc_pool.tile([P, 2 * NCLS], f32)  # DVE-accumulated S counts

        for b in range(BL):
            # ---- packed DMAs on one FIFO: q+halos, then 4 logit quarters
            qt = io_pool.tile([P, QW], bf16, tag="qt")
            nc.sync.dma_start(qt[:], q_ap[b])
            xq = []
            for qq in range(NQ):
                xt = io_pool.tile([P, NCLS, QF], fp8, tag=f"xt{qq}")
                nc.sync.dma_start(xt[:].rearrange("p c n -> p (c n)"), x_ap[b, qq])
                xq.append(xt)

            qf = qt[:, 0:FW]
            q3 = qf.rearrange("p (r w) -> p r w", r=R)
            hup = qt[:, FW : FW + W]
            hdn = qt[:, FW + W : FW + 2 * W]

            # ---- one-hots + S (DVE 4x) - ready as soon as q lands
            o = NCLS * b
            oh = work_pool.tile([P, NCLS, FW], bf16, tag="oh")
            for c in range(NCLS):
                nc.vector.tensor_scalar(
                    oh[:, c], qf, QVALS[c], None, op0=Alu.is_equal, op1=Alu.add,
                    accum_out=st_s[:, o + c : o + c + 1],
                )
            q4 = work_pool.tile([P, FW], bf16, tag="q4")
            nc.vector.tensor_scalar(q4[:], qf, 4.0, None, op0=Alu.mult)

            # ---- halo row sums: vertical on DVE (fast, keeps the combine
            # ready early), horizontal on Pool
            v = work_pool.tile([P, R, W], bf16, tag="v")
            nc.vector.tensor_tensor(
                v[:, 1:3, :], q3[:, 0:2, :], q3[:, 2:4, :], op=Alu.add
            )
            nc.vector.tensor_tensor(v[:, 0, :], hup, q3[:, 1, :], op=Alu.add)
            nc.vector.tensor_tensor(v[:, 3, :], q3[:, 2, :], hdn, op=Alu.add)
            h = work_pool.tile([P, FW], bf16, tag="h")
            nc.vector.memset(h[:, 0:1], SENTINEL)
            nc.vector.memset(h[:, FW - 1 : FW], SENTINEL)
            nc.gpsimd.tensor_tensor(
                h[:, 1 : FW - 1], qf[:, 0 : FW - 2], qf[:, 2:FW], op=Alu.add
            )

            ie = work_pool.tile([P, NCLS, FW], i16, tag="ie")
            s02 = work_pool.tile([P, 2, FW], bf16, tag="s02")
            se = work_pool.tile([P, FW], bf16, tag="se")
            nse = work_pool.tile([P, FW], i16, tag="nse")
            e = ie[:].bitcast(bf16)
            w = ie[:].bitcast(bf16)
            int_m = work_pool.tile([P, FW], bf16, tag="int_m")
            psum_i = psum_pool.tile([P, NCLS, 128], f32, tag="pi")
            psum_z = psum_pool.tile([P, NCLS, 128], f32, tag="pz")
            psum_n = psum_pool.tile([P, NCLS, 128], f32, tag="pn")
            cp = work_pool.tile([P, 3, NCLS * 128], bf16, tag="cp")

            def softmax_quarter(qq):
                qa = slice(qq * QF, (qq + 1) * QF)
                nc.scalar.activation(
                    ie[:, :, qa], xq[qq][:], AF.Copy, bias=B_EXP, scale=A_EXP
                )
                nc.vector.tensor_tensor(
                    s02[:, :, qa], e[:, 0:2, qa], e[:, 2:4, qa], op=Alu.add
                )
                nc.vector.tensor_tensor(
                    se[:, qa], s02[:, 0, qa], s02[:, 1, qa], op=Alu.add
                )
                nc.vector.tensor_scalar(
                    nse[:, qa], se[:, qa].bitcast(i16), -1.0, K_DIV,
                    op0=Alu.mult, op1=Alu.add,
                )
                nse_b = nse[:, qa].unsqueeze(1).broadcast_to((P, NCLS, QF))
                nc.vector.tensor_tensor(ie[:, :, qa], ie[:, :, qa], nse_b, op=Alu.add)

            def iz_quarter(qq):
                # one lazy-zero accumulation group per PSUM bank: only the
                # first matmul starts it, only the very last stops it. On the
                # final quarter run all I chains first so the I bounce+dump
                # overlap the Z chains.
                first = qq == 0
                last = qq == NQ - 1
                order = (
                    [(0, c) for c in range(NCLS)] + [(1, c) for c in range(NCLS)]
                    if last
                    else [(f, c) for c in range(NCLS) for f in (0, 1)]
                )
                for f, c in order:
                    ps = psum_i if f == 0 else psum_z
                    for ch in range(QF // 128):
                        sl = slice(qq * QF + ch * 128, qq * QF + (ch + 1) * 128)
                        nc.tensor.matmul(
                            ps[:, c], w[:, c, sl],
                            oh[:, c, sl] if f == 0 else w[:, c, sl],
                            start=(first and c == 0 and ch == 0),
                            stop=(last and c == NCLS - 1 and ch == QF // 128 - 1),
                        )
                    if last and f == 0 and c == NCLS - 1:
                        nc.scalar.activation(
                            cp[:, 0], psum_i[:].rearrange("p c n -> p (c n)"),
                            AF.Copy,
                        )
                        nc.sync.dma_start(dump_ap[b, :, 0 : NCLS * 128], cp[:, 0])

            for qq in range(NQ):
                softmax_quarter(qq)
                iz_quarter(qq)
                if qq == 1:
                    # interior mask combine on Pool, compare on DVE
                    nc.gpsimd.tensor_tensor(
                        h[:], h[:], v[:].rearrange("p r w -> p (r w)"), op=Alu.add
                    )
                    nc.vector.tensor_tensor(
                        int_m[:], h[:], q4[:], op=Alu.is_equal
                    )
                    i3 = int_m[:].rearrange("p (r w) -> p r w", r=R)
                    nc.vector.memset(i3[:, :, 0:1], 0.0)
                    nc.vector.memset(i3[:, :, W - 1 : W], 0.0)
                if qq == 1:
                    # N-chains as soon as the interior mask is ready
                    for c in range(NCLS):
                        for ch in range(FW // 128):
                            sl = slice(ch * 128, (ch + 1) * 128)
                            nc.tensor.matmul(
                                psum_n[:, c], oh[:, c, sl], int_m[:, sl],
                                start=(c == 0 and ch == 0),
                                stop=(c == NCLS - 1 and ch == FW // 128 - 1),
                            )
                    nc.scalar.activation(
                        cp[:, 2], psum_n[:].rearrange("p c n -> p (c n)"), AF.Copy
                    )
                    nc.sync.dma_start(
                        dump_ap[b, :, 2 * NCLS * 128 :], cp[:, 2]
                    )

            # Z bounce + dump trail (I was dumped inside the final quarter)
            nc.scalar.activation(
                cp[:, 1], psum_z[:].rearrange("p c n -> p (c n)"), AF.Copy
            )
            nc.sync.dma_start(
                dump_ap[b, :, NCLS * 128 : 2 * NCLS * 128], cp[:, 1]
            )

        nc.sync.dma_start(out_ap[:], st_s[:])


def _build():
    if "nc" in _cache:
        return _cache["nc"]
    nc = bacc.Bacc("TRN2", target_bir_lowering=False, debug=False, num_devices=N_CORES)
    x_ap = nc.dram_tensor("x", [BL, NQ, P, NCLS * QF], fp8, kind="ExternalInput").ap()
    q_ap = nc.dram_tensor("q", [BL, P, QW], bf16, kind="ExternalInput").ap()
    out_ap = nc.dram_tensor("stats", [P, 2 * NCLS], f32, kind="ExternalOutput").ap()
    dump_ap = nc.dram_tensor(
        "dumps", [BL, P, 3 * NCLS * 128], bf16, kind="ExternalOutput"
    ).ap()
    with tile.TileContext(nc) as tc:
        _kernel_body(nc, tc, x_ap, q_ap, out_ap, dump_ap)
    nc.compile()
    _cache["nc"] = nc
    return nc


def _host_inputs(inputs, target):
    """Pack per-core inputs into per-partition-contiguous layouts."""
    nb = inputs.shape[0]
    x8 = inputs.astype(ml_dtypes.float8_e4m3)
    # [b, c, (p qq), w] -> [b, qq, p, c, w]   (quarter qq = row 4p+qq)
    xp = np.ascontiguousarray(
        x8.reshape(nb, NCLS, P, NQ, W).transpose(0, 3, 2, 1, 4).reshape(
            nb, NQ, P, NCLS * QF
        )
    )
    q = np.power(4.0, target.astype(np.float32)).astype(ml_dtypes.bfloat16)
    qp = np.empty((nb, P, QW), dtype=ml_dtypes.bfloat16)
    qp[:, :, 0:FW] = q.reshape(nb, P, FW)
    qp[:, 1:, FW : FW + W] = q.reshape(nb, H, W)[:, R - 1 : H - 1 : R]  # hup
    qp[:, 0, FW : FW + W] = SENTINEL
    qp[:, : P - 1, FW + W :] = q.reshape(nb, H, W)[:, R : H : R]  # hdn
    qp[:, P - 1, FW + W :] = SENTINEL
    return xp, qp


_DIAG = np.arange(P)


def _diag_sums(dump):
    """dump: [P, 3*NCLS*128] -> [3, NCLS] diagonal sums."""
    d = dump.reshape(P, 3, NCLS, 128)
    izn = d[_DIAG, :, :, _DIAG].sum(axis=0)  # rows [I, Z, N]
    return izn[[0, 2, 1]]  # -> [I, N, Z]


def _finish(S, I, N, Z):
    C = S - N
    alpha = 1.0 - (C + SMOOTH) / (S + SMOOTH)
    alpha = np.minimum(2.0 * alpha - 1.0, 0.8)
    loss_c = (Z + S - 2.0 * I + SMOOTH) / (Z + S - (1.0 + alpha) * I + SMOOTH)
    return np.float32(loss_c.mean())


def kernel(inputs: np.ndarray, target: np.ndarray) -> np.ndarray:
    nc = _build()
    xp, qp = _host_inputs(inputs, target)
    in_maps = [
        {"x": xp[c * BL : (c + 1) * BL], "q": qp[c * BL : (c + 1) * BL]}
        for c in range(N_CORES)
    ]
    for attempt in range(3):
        res = run_bass_kernel_spmd(nc, in_maps, list(range(N_CORES)))
        S = np.zeros(NCLS)
        INZ = np.zeros((3, NCLS))
        ok = True
        for c in range(N_CORES):
            st = res.results[c]["stats"].astype(np.float64)
            dumps = res.results[c]["dumps"].astype(np.float64)
            ok &= bool(np.isfinite(st).all() and np.isfinite(dumps).all())
            S += st.sum(axis=0).reshape(BL, NCLS).sum(axis=0)
            for b in range(BL):
                INZ += _diag_sums(dumps[b])
        # S counts must equal the pixel total; retry on transient faults
        if ok and abs(S.sum() - B * H * W) < 0.5:
            break
    I, N, Z = INZ
    return _finish(S, I, N, Z)
